# revision 1
# baseline (speedup 1.0000x reference)
"""Trainium2 Bass kernel: ViT-style multimodal transformer (12L, D=768, H=12).

Strategy: pure data parallel — 8 batch elements, one per NeuronCore.
Each core runs the full transformer on its [667, 768] token sequence.

Device layouts (per core):
  - residual x:   SBUF [128, 6, 768] fp32, token t = j*128 + p  (natural: t on partitions)
  - hT/QT/KT/OT:  SBUF [128, 6, 667] bf16, feature-major (transposed: d on partitions,
                  t on free dim) — the layout matmul wants for both lhsT and rhs roles.
  - attention:    S^T[s,t] = K_h Q_h^T computed per head with s on partitions, exp on
                  ScalarE (no max subtraction; logits are tiny), denominator obtained by
                  appending a ones-column to V in the AV matmul, normalization applied to
                  O' via a K=1 broadcast matmul + multiply.
Token order is permuted (attention is permutation-equivariant; positional embeddings are
baked into the additive base): [obs(392) | goal(196) | cls | pose | text(77)], so patch
embeddings land partition-aligned. cls lives at row 588 = (j=4, p=76).
"""

import numpy as np
import ml_dtypes

import concourse.bass as bass
import concourse.bacc as bacc_mod
import concourse.mybir as mybir
import concourse.tile as tile
from concourse.bass_utils import run_bass_kernel_spmd
from concourse.masks import make_identity

BF16 = mybir.dt.bfloat16
F32 = mybir.dt.float32
AF = mybir.ActivationFunctionType

L, H, D, HD = 12, 12, 768, 64
P, IMG, NP, HS = 16, 224, 196, 2
TBLK, VOCAB, POSE_DIM, OUT = 77, 96, 7, 7
B = 8
SEQ = 667          # 1 cls + 1 pose + 392 obs + 77 text + 196 goal
TPAD = 768         # padded token slots (6 partition tiles)
NT = 6             # token partition tiles
ND = 6             # feature partition tiles (768/128)
NF = 24            # ffn feature tiles (3072/128)
SCALE = float(D) ** -0.5
EPS = 1e-5

# token tiles (start, width)
TT = [(0, 128), (128, 128), (256, 128), (384, 128), (512, 128), (640, 27)]


def _chunks(total, cap=512):
    s = 0
    out = []
    while s < total:
        w = min(cap, total - s)
        out.append((s, w))
        s += w
    return out


CH_T = _chunks(SEQ)    # [(0,512),(512,155)]
CH_D = _chunks(D)      # [(0,512),(512,256)]

# Runtime knobs (test.py may flip these)
TRACE = False
TRACE_CORES = [0]
LAST_EXEC_NS = None
_CACHE = {}


def _bcast128(ap1d):
    """DMA access pattern broadcasting a 1-D DRAM row across 128 partitions."""
    return bass.AP(tensor=ap1d.tensor, offset=ap1d.offset,
                   ap=[[0, 128]] + list(ap1d.ap))


def build_nc():
    nc = bacc_mod.Bacc()

    # ---- per-core data inputs ----
    base = nc.declare_dram_parameter("base", [TPAD, D], F32, isOutput=False)
    pobsT = nc.declare_dram_parameter("pobsT", [D, 392], BF16, isOutput=False)
    pgoalT = nc.declare_dram_parameter("pgoalT", [D, 204], BF16, isOutput=False)
    # ---- shared weights ----
    obs_w = nc.declare_dram_parameter("obs_w", [D, D], BF16, isOutput=False)
    goal_w = nc.declare_dram_parameter("goal_w", [D, D], BF16, isOutput=False)
    wq = nc.declare_dram_parameter("wq", [L, D, D], BF16, isOutput=False)
    wk = nc.declare_dram_parameter("wk", [L, D, D], BF16, isOutput=False)
    wv = nc.declare_dram_parameter("wv", [L, D, D], BF16, isOutput=False)
    pw = nc.declare_dram_parameter("pw", [L, D, D], BF16, isOutput=False)
    fw1 = nc.declare_dram_parameter("fw1", [L, D, 4 * D], BF16, isOutput=False)
    fw2 = nc.declare_dram_parameter("fw2", [L, 4 * D, D], BF16, isOutput=False)
    pb = nc.declare_dram_parameter("pb", [L, D], F32, isOutput=False)
    fb1 = nc.declare_dram_parameter("fb1", [L, 4 * D], F32, isOutput=False)
    fb2 = nc.declare_dram_parameter("fb2", [L, D], F32, isOutput=False)
    ln1g = nc.declare_dram_parameter("ln1g", [L, D], F32, isOutput=False)
    ln1b = nc.declare_dram_parameter("ln1b", [L, D], F32, isOutput=False)
    ln2g = nc.declare_dram_parameter("ln2g", [L, D], F32, isOutput=False)
    ln2b = nc.declare_dram_parameter("ln2b", [L, D], F32, isOutput=False)
    clsout = nc.declare_dram_parameter("clsout", [1, D], F32, isOutput=True)

    with tile.TileContext(nc) as tc:
        with (
            tc.tile_pool(name="singles", bufs=1) as singles,
            tc.tile_pool(name="lnv", bufs=2) as lnv,
            tc.tile_pool(name="wblk", bufs=6) as wblk,
            tc.tile_pool(name="rhsk", bufs=6) as rhsk,
            tc.tile_pool(name="rows", bufs=2) as rows,
            tc.tile_pool(name="hn", bufs=3) as hn,
            tc.tile_pool(name="upool", bufs=2) as upool,
            tc.tile_pool(name="tmpo", bufs=2) as tmpo,
            tc.tile_pool(name="stats", bufs=6) as stats,
            tc.tile_pool(name="pbig", bufs=2, space="PSUM") as pbig,
            tc.tile_pool(name="patt", bufs=2, space="PSUM") as patt,
        ):
            # ---------- persistent SBUF ----------
            ident = singles.tile([128, 128], BF16)
            make_identity(nc, ident)
            eps_sb = singles.tile([128, 1], F32)
            nc.vector.memset(eps_sb, EPS)
            ones_sb = singles.tile([1, 128], F32)
            nc.vector.memset(ones_sb, 1.0)

            x = singles.tile([128, NT, D], F32)            # residual stream
            hT = singles.tile([128, ND, SEQ], BF16)        # LN output, transposed
            QT = singles.tile([128, ND, SEQ], BF16)
            KT = singles.tile([128, ND, SEQ], BF16)
            vbuf = singles.tile([128, NT, H, HD + 1], BF16)  # V natural + ones col
            OT = singles.tile([128, ND, SEQ], BF16)        # attn out, transposed
            h3T = singles.tile([128, NF, SEQ], BF16)       # relu ffn hidden, transposed
            rbuf = singles.tile([1, H, SEQ], F32)          # per-head 1/denominator

            nc.vector.memset(vbuf[:, :, :, HD:HD + 1], 1.0)

            # ---------- load residual base ----------
            nc.sync.dma_start(out=x[:], in_=base.rearrange("(j p) d -> p j d", p=128))

            # ---------- patch embeddings ----------
            pobs_sb = singles.tile([128, ND, 392], BF16)
            nc.sync.dma_start(out=pobs_sb[:],
                              in_=pobsT.rearrange("(kt kp) t -> kp kt t", kp=128))
            pgoal_sb = singles.tile([128, ND, 204], BF16)
            nc.sync.dma_start(out=pgoal_sb[:],
                              in_=pgoalT.rearrange("(kt kp) t -> kp kt t", kp=128))

            def embed_add(psrc_sb, w_dram, ptiles, dests):
                # ptiles: list of (col0, width); dests: list of (xrow0, xj)
                for gi in range(0, len(ptiles), 2):
                    grp = list(range(gi, min(gi + 2, len(ptiles))))
                    psums = {}
                    for t_i in grp:
                        psums[t_i] = pbig.tile([128, D], F32, tag="pbig", name=f"ps{t_i}")
                    for k in range(ND):
                        wk_t = rhsk.tile([128, D], BF16, tag="rhsk")
                        nc.gpsimd.dma_start(out=wk_t[:], in_=w_dram[k * 128:(k + 1) * 128, :])
                        for t_i in grp:
                            c0, cw = ptiles[t_i]
                            for (s, w) in CH_D:
                                nc.tensor.matmul(
                                    psums[t_i][:cw, s:s + w],
                                    lhsT=psrc_sb[:, k, c0:c0 + cw],
                                    rhs=wk_t[:, s:s + w],
                                    start=(k == 0), stop=(k == ND - 1))
                    for t_i in grp:
                        c0, cw = ptiles[t_i]
                        r0, xj = dests[t_i]
                        nc.vector.tensor_add(out=x[r0:r0 + cw, xj, :],
                                             in0=x[r0:r0 + cw, xj, :],
                                             in1=psums[t_i][:cw, :])

            embed_add(pobs_sb, obs_w,
                      [(0, 128), (128, 128), (256, 128), (384, 8)],
                      [(0, 0), (0, 1), (0, 2), (0, 3)])
            embed_add(pgoal_sb, goal_w,
                      [(0, 128), (128, 76)],
                      [(0, 3), (0, 4)])

            # ---------- helpers ----------
            def layer_norm_into_hT(g_dram, b_dram):
                g_bc = lnv.tile([128, D], F32, tag="g")
                b_bc = lnv.tile([128, D], F32, tag="b")
                nc.sync.dma_start(out=g_bc[:], in_=_bcast128(g_dram))
                nc.sync.dma_start(out=b_bc[:], in_=_bcast128(b_dram))
                for ti, (t0, tw) in enumerate(TT):
                    st = stats.tile([128, 3, 6], F32, tag="bnst")
                    mv = stats.tile([128, 2], F32, tag="bnmv")
                    rstd = stats.tile([128, 1], F32, tag="rstd")
                    xi = x[:tw, ti, :].rearrange("p (s c) -> p s c", s=3)
                    for s in range(3):
                        nc.vector.bn_stats(out=st[:tw, s, :], in_=xi[:, s, :])
                    nc.vector.bn_aggr(out=mv[:tw], in_=st[:tw])
                    nc.scalar.activation(out=rstd[:tw], in_=mv[:tw, 1:2],
                                         func=AF.Sqrt, bias=eps_sb[:tw], scale=1.0)
                    nc.vector.reciprocal(out=rstd[:tw], in_=rstd[:tw])
                    hpre = hn.tile([128, D], F32, tag="hpre")
                    nc.vector.tensor_scalar(out=hpre[:tw], in0=x[:tw, ti, :],
                                            scalar1=mv[:tw, 0:1], scalar2=rstd[:tw],
                                            op0=mybir.AluOpType.subtract,
                                            op1=mybir.AluOpType.mult)
                    nc.vector.tensor_mul(out=hpre[:tw], in0=hpre[:tw], in1=g_bc[:tw])
                    hnat = hn.tile([128, D], BF16, tag="hnat")
                    nc.vector.tensor_add(out=hnat[:tw], in0=hpre[:tw], in1=b_bc[:tw])
                    # transpose into hT
                    for dj in range(ND):
                        pt = patt.tile([128, SEQ], BF16, tag="patt")
                        nc.tensor.transpose(pt[:, :tw], hnat[:tw, dj * 128:(dj + 1) * 128],
                                            ident[:tw, :tw])
                        nc.any.tensor_copy(out=hT[:, dj, t0:t0 + tw], in_=pt[:, :tw])

            def linear_T(w_dram, out_sb, n_tiles, src_sb, src_ntiles, bias_row=None,
                         relu=False):
                """out_sb[:, n, t] (transposed layout) = w.T @ src ( + bias, relu )."""
                for n in range(n_tiles):
                    wb = wblk.tile([128, src_ntiles, 128], BF16, tag="wblk")
                    nc.gpsimd.dma_start(
                        out=wb[:],
                        in_=w_dram.rearrange("(kt kp) n -> kp kt n", kp=128)
                        [:, :, n * 128:(n + 1) * 128])
                    ps = pbig.tile([128, D], F32, tag="pbig")
                    for k in range(src_ntiles):
                        for (s, w) in CH_T:
                            nc.tensor.matmul(ps[:, s:s + w],
                                             lhsT=wb[:, k, :],
                                             rhs=src_sb[:, k, s:s + w],
                                             start=(k == 0), stop=(k == src_ntiles - 1))
                    if relu:
                        nc.scalar.activation(out=out_sb[:, n, :], in_=ps[:, :SEQ],
                                             func=AF.Relu, bias=bias_row[:, n:n + 1],
                                             scale=1.0)
                    else:
                        nc.vector.tensor_copy(out=out_sb[:, n, :], in_=ps[:, :SEQ])

            def linear_N(w_dram, k_tiles, src_sb, bias_row, dest_residual=True):
                """natural-layout output accumulated into x: x += src.T@w + b."""
                for gi in range(0, NT, 2):
                    grp = [g for g in range(gi, min(gi + 2, NT))]
                    psums = {}
                    for t_i in grp:
                        psums[t_i] = pbig.tile([128, D], F32, tag="pbig", name=f"ps{t_i}")
                    for k in range(k_tiles):
                        wk_t = rhsk.tile([128, D], BF16, tag="rhsk")
                        nc.gpsimd.dma_start(out=wk_t[:],
                                            in_=w_dram[k * 128:(k + 1) * 128, :])
                        for t_i in grp:
                            t0, tw = TT[t_i]
                            for (s, w) in CH_D:
                                nc.tensor.matmul(psums[t_i][:tw, s:s + w],
                                                 lhsT=src_sb[:, k, t0:t0 + tw],
                                                 rhs=wk_t[:, s:s + w],
                                                 start=(k == 0), stop=False)
                    for t_i in grp:
                        t0, tw = TT[t_i]
                        # += bias via K=1 ones matmul, closes the accumulation group
                        for (s, w) in CH_D:
                            nc.tensor.matmul(psums[t_i][:tw, s:s + w],
                                             lhsT=ones_sb[0:1, :tw],
                                             rhs=bias_row[0:1, s:s + w],
                                             start=False, stop=True)
                        nc.vector.tensor_add(out=x[:tw, t_i, :], in0=x[:tw, t_i, :],
                                             in1=psums[t_i][:tw, :])

            # ---------- transformer layers ----------
            for l in range(L):
                # LN1 -> hT
                layer_norm_into_hT(ln1g[l], ln1b[l])

                # QT, KT
                linear_T(wq[l], QT, ND, hT, ND)
                linear_T(wk[l], KT, ND, hT, ND)

                # V natural into vbuf (+ ones col preset)
                for gi in range(0, NT, 2):
                    grp = [g for g in range(gi, min(gi + 2, NT))]
                    psums = {}
                    for t_i in grp:
                        psums[t_i] = pbig.tile([128, D], F32, tag="pbig", name=f"ps{t_i}")
                    for k in range(ND):
                        wk_t = rhsk.tile([128, D], BF16, tag="rhsk")
                        nc.gpsimd.dma_start(out=wk_t[:],
                                            in_=wv[l][k * 128:(k + 1) * 128, :])
                        for t_i in grp:
                            t0, tw = TT[t_i]
                            for (s, w) in CH_D:
                                nc.tensor.matmul(psums[t_i][:tw, s:s + w],
                                                 lhsT=hT[:, k, t0:t0 + tw],
                                                 rhs=wk_t[:, s:s + w],
                                                 start=(k == 0), stop=(k == ND - 1))
                    for t_i in grp:
                        t0, tw = TT[t_i]
                        nc.vector.tensor_copy(
                            out=vbuf[:tw, t_i, :, 0:HD],
                            in_=psums[t_i][:tw, :].rearrange("p (h d) -> p h d", h=H))

                # attention per head
                def emit_ST(h):
                    j, r = h // 2, (h % 2) * 64
                    u = upool.tile([128, NT, SEQ], BF16, tag="U")
                    for s_i, (s0, sw) in enumerate(TT):
                        ps = patt.tile([128, SEQ], F32, tag="patt")
                        for (c, w) in CH_T:
                            nc.tensor.matmul(ps[:sw, c:c + w],
                                             lhsT=KT[r:r + 64, j, s0:s0 + sw],
                                             rhs=QT[r:r + 64, j, c:c + w],
                                             start=True, stop=True)
                        nc.scalar.activation(out=u[:sw, s_i, :], in_=ps[:sw, :SEQ],
                                             func=AF.Exp, scale=SCALE)
                    return u

                def emit_AV(h, u):
                    j, r = h // 2, (h % 2) * 64
                    po = pbig.tile([128, D], F32, tag="pbig")  # use [:65, :SEQ]
                    for s_i, (s0, sw) in enumerate(TT):
                        for (c, w) in CH_T:
                            nc.tensor.matmul(po[:HD + 1, c:c + w],
                                             lhsT=vbuf[:sw, s_i, h, :],
                                             rhs=u[:sw, s_i, c:c + w],
                                             start=(s_i == 0), stop=(s_i == NT - 1))
                    nc.vector.reciprocal(out=rbuf[0:1, h, :], in_=po[HD:HD + 1, :SEQ])
                    pbc = pbig.tile([128, D], F32, tag="pbig")  # use [:64, :SEQ]
                    for (c, w) in CH_T:
                        nc.tensor.matmul(pbc[:HD, c:c + w],
                                         lhsT=ones_sb[0:1, :HD],
                                         rhs=rbuf[0:1, h, c:c + w],
                                         start=True, stop=True)
                    to = tmpo.tile([HD, SEQ], BF16, tag="tmpo")
                    nc.scalar.activation(out=to[:], in_=po[:HD, :SEQ], func=AF.Copy)
                    nc.vector.tensor_mul(out=OT[r:r + 64, j, :], in0=to[:],
                                         in1=pbc[:HD, :SEQ])

                u_prev = emit_ST(0)
                for h in range(1, H):
                    u_cur = emit_ST(h)
                    emit_AV(h - 1, u_prev)
                    u_prev = u_cur
                emit_AV(H - 1, u_prev)

                # proj + residual
                pb_row = rows.tile([1, D], F32, tag="row")
                nc.sync.dma_start(out=pb_row[:], in_=pb[l][None, :])
                linear_N(pw[l], ND, OT, pb_row)

                # LN2 -> hT
                layer_norm_into_hT(ln2g[l], ln2b[l])

                # FFN
                fb1_sb = rows.tile([128, NF], F32, tag="fb1")
                nc.sync.dma_start(out=fb1_sb[:],
                                  in_=fb1[l].rearrange("(t p) -> p t", p=128))
                linear_T(fw1[l], h3T, NF, hT, ND, bias_row=fb1_sb, relu=True)
                fb2_row = rows.tile([1, D], F32, tag="row")
                nc.sync.dma_start(out=fb2_row[:], in_=fb2[l][None, :])
                linear_N(fw2[l], NF, h3T, fb2_row)

            # ---------- output: cls residual row (row 588 = j4, p76) ----------
            nc.sync.dma_start(out=clsout[:, :], in_=x[76:77, 4, :])

    nc.finalize()
    return nc


# ======================= host side =======================

def _sincos_pos(T, d):
    i = np.arange(T, dtype=np.float64)[:, None]
    j = np.arange(d, dtype=np.float64)[None, :]
    je = np.where(j % 2 == 0, j, j - 1)
    ang = i / np.power(10000.0, je / d)
    pe = np.where(j % 2 == 0, np.sin(ang), np.cos(ang))
    return pe.astype(np.float32)


def _patchify_stacked(img):
    b = img.shape[0]
    x = img.reshape(b, IMG // P, P, IMG // P, P, 3, HS)
    x = x.transpose(0, 1, 3, 6, 2, 4, 5)
    return x.reshape(b, NP * HS, P * P * 3)


def _patchify3(img):
    b = img.shape[0]
    x = img.reshape(b, IMG // P, P, IMG // P, P, 3)
    x = x.transpose(0, 1, 3, 2, 4, 5)
    return x.reshape(b, NP, P * P * 3)


def _layernorm_np(v, g, b, eps=1e-5):
    m = v.mean(axis=-1, keepdims=True)
    s = v.var(axis=-1, keepdims=True)
    return (v - m) / np.sqrt(s + eps) * g + b


PERM = np.concatenate([np.arange(2, 394), np.arange(471, 667),
                       np.array([0, 1]), np.arange(394, 471)])


def kernel(**inputs):
    global LAST_EXEC_NS
    f32 = lambda k: np.asarray(inputs[k], dtype=np.float32)
    bf = lambda a: np.ascontiguousarray(np.asarray(a, dtype=np.float32)
                                        .astype(ml_dtypes.bfloat16))

    if "nc" not in _CACHE:
        _CACHE["nc"] = build_nc()
    nc = _CACHE["nc"]

    images = f32("images")
    goal_imgs = f32("goal_imgs")
    pose = f32("pose")
    txt = np.asarray(inputs["goals_txt"]).astype(np.int64)
    tok_emb = f32("tok_emb")

    # pose MLP (host, exact fp32 – 4.7 MFLOP)
    pose_tok = np.maximum(pose @ f32("pose_w1") + f32("pose_b1"), 0.0) \
        @ f32("pose_w2") + f32("pose_b2")                       # [B, D]

    pos = _sincos_pos(SEQ, D)                                    # [667, D]
    content = np.zeros((B, SEQ, D), np.float32)
    content[:, 0, :] = f32("cls_tok")[0, 0]
    content[:, 1, :] = pose_tok
    content[:, 2:394, :] = f32("obs_b")
    content[:, 394:471, :] = tok_emb[txt]
    content[:, 471:667, :] = f32("goal_b")
    base = (content + pos[None])[:, PERM, :]                     # permuted
    base_pad = np.zeros((B, TPAD, D), np.float32)
    base_pad[:, :SEQ, :] = base

    p_obs = _patchify_stacked(images)                            # [B, 392, 768]
    p_goal = _patchify3(goal_imgs)                               # [B, 196, 768]
    pobsT = bf(p_obs.transpose(0, 2, 1))                         # [B, 768, 392]
    pgoalT_np = np.zeros((B, D, 204), np.float32)
    pgoalT_np[:, :, 8:] = p_goal.transpose(0, 2, 1)
    pgoalT = bf(pgoalT_np)

    shared = {
        "obs_w": bf(f32("obs_w")), "goal_w": bf(f32("goal_w")),
        "wq": bf(f32("wq")), "wk": bf(f32("wk")), "wv": bf(f32("wv")),
        "pw": bf(f32("proj_w")), "fw1": bf(f32("ff_w1")), "fw2": bf(f32("ff_w2")),
        "pb": f32("proj_b"), "fb1": f32("ff_b1"), "fb2": f32("ff_b2"),
        "ln1g": f32("ln1_g"), "ln1b": f32("ln1_b"),
        "ln2g": f32("ln2_g"), "ln2b": f32("ln2_b"),
    }
    in_maps = []
    for b in range(B):
        m = dict(shared)
        m["base"] = np.ascontiguousarray(base_pad[b])
        m["pobsT"] = np.ascontiguousarray(pobsT[b])
        m["pgoalT"] = np.ascontiguousarray(pgoalT[b])
        in_maps.append(m)

    res = run_bass_kernel_spmd(nc, in_maps, list(range(B)), trace=TRACE,
                               trace_cores=TRACE_CORES if TRACE else None)
    LAST_EXEC_NS = res.exec_time_ns

    cls = np.stack([np.asarray(res.results[b]["clsout"][0], np.float32)
                    for b in range(B)])                          # [B, D]
    h = _layernorm_np(cls, f32("lnf_g"), f32("lnf_b"))
    h = _layernorm_np(h, f32("hln_g"), f32("hln_b"))
    out = h @ f32("head_w") + f32("head_b")
    return out.astype(np.float32)



# revision 12
# speedup vs baseline: 1.3001x; 1.3001x over previous
"""Trainium2 Bass kernel: ViT-style multimodal transformer (12L, D=768, H=12).

Strategy: pure data parallel - 8 batch elements, one per NeuronCore.
Each core runs the full transformer on its [667, 768] token sequence.

Optimizations over the v0 baseline (all bf16 - fp8 fails the 2e-2 gate):
  - Attention S^T computed as row-tiled head PAIRS: even head uses PE rows
    0-63, odd head rows 64-127 (tile_position auto-derived from
    base_partition), so the two K=64 matmuls run concurrently -> S^T cost
    halves.
  - Softmax denominator (ones-column of V) broadcast with a K=1 BF16
    ones-matmul of the raw denominator row (the baseline broadcast fp32
    reciprocals), then a 64-lane reciprocal; normalization fused into the
    O^T evacuation (one scalar_tensor_tensor op). Kills the 2.2us
    single-partition reciprocals of the baseline.
  - Residual adds fused: x = psum + x in one DVE op (no separate copies,
    no K=1 fp32 bias matmuls; biases/LN gains only emitted when nonzero).
  - Last layer computes attention/proj/FFN for the cls token only.

Token order is permuted (attention is permutation-equivariant; positional
embeddings are baked into the additive base): [obs(392) | goal(196) | cls |
pose | text(77)], so patch embeddings land partition-aligned. cls lives at
row 588 = (j=4, p=76).
"""

import numpy as np
import ml_dtypes

import concourse.bass as bass
import concourse.bacc as bacc_mod
import concourse.mybir as mybir
import concourse.tile as tile
from concourse.bass_utils import run_bass_kernel_spmd
from concourse.masks import make_identity

BF16 = mybir.dt.bfloat16
F32 = mybir.dt.float32
AF = mybir.ActivationFunctionType
ALU = mybir.AluOpType

L, H, D, HD = 12, 12, 768, 64
P, IMG, NP, HS = 16, 224, 196, 2
TBLK, VOCAB, POSE_DIM, OUT = 77, 96, 7, 7
B = 8
SEQ = 667          # 1 cls + 1 pose + 392 obs + 77 text + 196 goal
TPAD = 768         # padded token slots (6 partition tiles)
SPAD = 672         # padded free-dim length of transposed activations
NT = 6             # token partition tiles
ND = 6             # feature partition tiles (768/128)
NF = 24            # ffn feature tiles (3072/128)
SCALE = float(D) ** -0.5
EPS = 1e-5
CLS = 588          # permuted cls position = (tile 4, row 76)
CLS_J, CLS_P = 4, 76

# token tiles (start, width)
TT = [(0, 128), (128, 128), (256, 128), (384, 128), (512, 128), (640, 27)]
CH_T = [(0, 512), (512, 155)]   # SEQ chunks (psum bank = 512 fp32)
CH_D = [(0, 512), (512, 256)]   # D chunks
CH_CLS = [(CLS, 1)]             # cls-only chunk (last layer)

# Runtime knobs (test.py may flip these)
TRACE = False
TRACE_CORES = [0]
CLS_LAST = True
LAST_EXEC_NS = None
_CACHE = {}


def _bcast128(ap1d):
    """DMA access pattern broadcasting a 1-D DRAM row across 128 partitions."""
    return bass.AP(tensor=ap1d.tensor, offset=ap1d.offset,
                   ap=[[0, 128]] + list(ap1d.ap))


def build_nc(has_gb=False, has_bias=False, layers=L, cls_last=True):
    nc = bacc_mod.Bacc()

    # ---- per-core data inputs ----
    base = nc.declare_dram_parameter("base", [TPAD, D], F32, isOutput=False)
    pobsT = nc.declare_dram_parameter("pobsT", [D, 392], BF16, isOutput=False)
    pgoalT = nc.declare_dram_parameter("pgoalT", [D, 204], BF16, isOutput=False)
    # ---- shared weights ----
    obs_w = nc.declare_dram_parameter("obs_w", [D, D], BF16, isOutput=False)
    goal_w = nc.declare_dram_parameter("goal_w", [D, D], BF16, isOutput=False)
    wq = nc.declare_dram_parameter("wq", [L, D, D], BF16, isOutput=False)
    wk = nc.declare_dram_parameter("wk", [L, D, D], BF16, isOutput=False)
    wv = nc.declare_dram_parameter("wv", [L, D, D], BF16, isOutput=False)
    pw = nc.declare_dram_parameter("pw", [L, D, D], BF16, isOutput=False)
    fw1 = nc.declare_dram_parameter("fw1", [L, D, 4 * D], BF16, isOutput=False)
    fw2 = nc.declare_dram_parameter("fw2", [L, 4 * D, D], BF16, isOutput=False)
    if has_bias:
        pb = nc.declare_dram_parameter("pb", [L, D], F32, isOutput=False)
        fb1 = nc.declare_dram_parameter("fb1", [L, 4 * D], F32, isOutput=False)
        fb2 = nc.declare_dram_parameter("fb2", [L, D], F32, isOutput=False)
    if has_gb:
        ln1g = nc.declare_dram_parameter("ln1g", [L, D], F32, isOutput=False)
        ln1b = nc.declare_dram_parameter("ln1b", [L, D], F32, isOutput=False)
        ln2g = nc.declare_dram_parameter("ln2g", [L, D], F32, isOutput=False)
        ln2b = nc.declare_dram_parameter("ln2b", [L, D], F32, isOutput=False)
    clsout = nc.declare_dram_parameter("clsout", [1, D], F32, isOutput=True)

    with tile.TileContext(nc) as tc:
        with (
            tc.tile_pool(name="singles", bufs=1) as singles,
            tc.tile_pool(name="wblk", bufs=4) as wblk,    # lin_T weight tiles
            tc.tile_pool(name="rhsk", bufs=6) as rhsk,    # lin_N weight k-tiles
            tc.tile_pool(name="upool", bufs=4) as upool,  # exp(S^T) per head
            tc.tile_pool(name="hn", bufs=2) as hn,
            tc.tile_pool(name="rows", bufs=2) as rows,
            tc.tile_pool(name="stats", bufs=6) as stats,
            tc.tile_pool(name="lnv", bufs=4) as lnv,
            tc.tile_pool(name="pp", bufs=8, space="PSUM") as pp,
        ):
            # ---------- persistent SBUF ----------
            ident = singles.tile([128, 128], BF16)
            make_identity(nc, ident)
            eps_sb = singles.tile([128, 1], F32)
            nc.vector.memset(eps_sb, EPS)
            ones_sb = singles.tile([1, 128], BF16)
            nc.vector.memset(ones_sb, 1.0)

            x = singles.tile([128, NT, D], F32)            # residual stream
            hT = singles.tile([128, ND, SPAD], BF16)       # LN output, transposed
            QT = singles.tile([128, ND, SPAD], BF16)
            KT = singles.tile([128, ND, SPAD], BF16)
            vbuf = singles.tile([128, NT, H, HD + 1], BF16)  # V natural + ones col
            OT = singles.tile([128, ND, SPAD], BF16)       # attn out, transposed
            h3T = singles.tile([128, NF, SPAD], BF16)      # relu ffn hidden, transposed

            nc.vector.memset(vbuf[:, :, :, HD:HD + 1], 1.0)

            # ---------- load residual base ----------
            nc.sync.dma_start(out=x[:], in_=base.rearrange("(j p) d -> p j d", p=128))

            # ---------- patch embeddings ----------
            pobs_sb = singles.tile([128, ND, 392], BF16)
            nc.sync.dma_start(out=pobs_sb[:],
                              in_=pobsT.rearrange("(kt kp) t -> kp kt t", kp=128))
            pgoal_sb = singles.tile([128, ND, 204], BF16)
            nc.sync.dma_start(out=pgoal_sb[:],
                              in_=pgoalT.rearrange("(kt kp) t -> kp kt t", kp=128))

            def embed_add(psrc_sb, w_dram, ptiles, dests):
                # ptiles: list of (col0, width); dests: list of (xrow0, xj)
                for gi in range(0, len(ptiles), 2):
                    grp = list(range(gi, min(gi + 2, len(ptiles))))
                    psums = {}
                    for t_i in grp:
                        psums[t_i] = [pp.tile([128, 512], F32, tag="b",
                                              name=f"pe{t_i}{ci}")
                                      for ci in range(len(CH_D))]
                    for k in range(ND):
                        wk_t = rhsk.tile([128, D], BF16, tag="rhsk")
                        nc.gpsimd.dma_start(out=wk_t[:],
                                            in_=w_dram[k * 128:(k + 1) * 128, :])
                        for t_i in grp:
                            c0, cw = ptiles[t_i]
                            for ci, (s, w) in enumerate(CH_D):
                                nc.tensor.matmul(
                                    psums[t_i][ci][:cw, :w],
                                    lhsT=psrc_sb[:, k, c0:c0 + cw],
                                    rhs=wk_t[:, s:s + w],
                                    start=(k == 0), stop=(k == ND - 1))
                    for t_i in grp:
                        c0, cw = ptiles[t_i]
                        r0, xj = dests[t_i]
                        for ci, (s, w) in enumerate(CH_D):
                            nc.vector.tensor_add(
                                out=x[r0:r0 + cw, xj, s:s + w],
                                in0=x[r0:r0 + cw, xj, s:s + w],
                                in1=psums[t_i][ci][:cw, :w])

            embed_add(pobs_sb, obs_w,
                      [(0, 128), (128, 128), (256, 128), (384, 8)],
                      [(0, 0), (0, 1), (0, 2), (0, 3)])
            embed_add(pgoal_sb, goal_w,
                      [(0, 128), (128, 76)],
                      [(0, 3), (0, 4)])

            # ---------- helpers ----------
            def layer_norm_tile(ti, t0, tw, g_bc, b_bc):
                st = stats.tile([128, 3, 6], F32, tag="bnst")
                mv = stats.tile([128, 2], F32, tag="bnmv")
                rstd = stats.tile([128, 1], F32, tag="rstd")
                xi = x[:tw, ti, :].rearrange("p (s c) -> p s c", s=3)
                for s in range(3):
                    nc.vector.bn_stats(out=st[:tw, s, :], in_=xi[:, s, :])
                nc.vector.bn_aggr(out=mv[:tw], in_=st[:tw])
                nc.scalar.activation(out=rstd[:tw], in_=mv[:tw, 1:2],
                                     func=AF.Sqrt, bias=eps_sb[:tw], scale=1.0)
                nc.vector.reciprocal(out=rstd[:tw], in_=rstd[:tw])
                hnat = hn.tile([128, D], BF16, tag="hnat")
                nc.vector.tensor_scalar(out=hnat[:tw], in0=x[:tw, ti, :],
                                        scalar1=mv[:tw, 0:1], scalar2=rstd[:tw],
                                        op0=ALU.subtract, op1=ALU.mult)
                if has_gb:
                    nc.vector.tensor_mul(out=hnat[:tw], in0=hnat[:tw], in1=g_bc[:tw])
                    nc.vector.tensor_add(out=hnat[:tw], in0=hnat[:tw], in1=b_bc[:tw])
                # transpose into hT
                for dj in range(ND):
                    pt = pp.tile([128, 128], BF16, tag="b", name="pt")
                    nc.tensor.transpose(pt[:, :tw],
                                        hnat[:tw, dj * 128:(dj + 1) * 128],
                                        ident[:tw, :tw])
                    nc.scalar.activation(out=hT[:, dj, t0:t0 + tw],
                                         in_=pt[:, :tw], func=AF.Copy)

            def layer_norm_into_hT(g_dram=None, b_dram=None, tiles=None):
                g_bc = b_bc = None
                if has_gb:
                    g_bc = lnv.tile([128, D], F32, tag="g")
                    b_bc = lnv.tile([128, D], F32, tag="bb")
                    nc.sync.dma_start(out=g_bc[:], in_=_bcast128(g_dram))
                    nc.sync.dma_start(out=b_bc[:], in_=_bcast128(b_dram))
                for ti, (t0, tw) in enumerate(TT):
                    if tiles is not None and ti not in tiles:
                        continue
                    layer_norm_tile(ti, t0, tw, g_bc, b_bc)

            def lin_T_n(w_dram_l, out_sb, n, src=None, relu=False, bias_col=None,
                        ch=CH_T):
                """One n-tile of a transposed-output linear: out[:, n, t]."""
                if src is None:
                    src = hT
                k_tiles = src.shape[1]
                wb = wblk.tile([128, k_tiles, 128], BF16, tag="wblk")
                nc.gpsimd.dma_start(
                    out=wb[:],
                    in_=w_dram_l.rearrange("(kt kp) n -> kp kt n", kp=128)
                    [:, :, n * 128:(n + 1) * 128])
                pss = [pp.tile([128, 512], F32, tag="b", name=f"lt{ci}")
                       for ci in range(len(ch))]
                for k in range(k_tiles):
                    for ci, (c0, w) in enumerate(ch):
                        nc.tensor.matmul(
                            pss[ci][:, :w],
                            lhsT=wb[:, k, :],
                            rhs=src[:, k, c0:c0 + w],
                            start=(k == 0), stop=(k == k_tiles - 1))
                for ci, (c0, w) in enumerate(ch):
                    if relu:
                        bias = bias_col[:, n:n + 1] if bias_col is not None else 0.0
                        nc.scalar.activation(out=out_sb[:, n, c0:c0 + w],
                                             in_=pss[ci][:, :w], func=AF.Relu,
                                             bias=bias, scale=1.0)
                    else:
                        nc.scalar.activation(out=out_sb[:, n, c0:c0 + w],
                                             in_=pss[ci][:, :w], func=AF.Copy)

            def lin_N(w_dram_l, src_sb, k_tiles, evac, tiles=None):
                """Natural-layout output: psum[t, 0:768] = src.T @ w per token tile."""
                tlist = [(t_i, t0, tw) for t_i, (t0, tw) in enumerate(TT)
                         if tiles is None or t_i in tiles]
                for gi in range(0, len(tlist), 2):
                    grp = tlist[gi:gi + 2]
                    psums = {}
                    for (t_i, t0, tw) in grp:
                        psums[t_i] = [pp.tile([128, 512], F32, tag="b",
                                              name=f"ln{t_i}{ci}")
                                      for ci in range(len(CH_D))]
                    for k in range(k_tiles):
                        wk_t = rhsk.tile([128, D], BF16, tag="rhsk")
                        nc.gpsimd.dma_start(out=wk_t[:],
                                            in_=w_dram_l[k * 128:(k + 1) * 128, :])
                        for (t_i, t0, tw) in grp:
                            for ci, (c0, w) in enumerate(CH_D):
                                nc.tensor.matmul(
                                    psums[t_i][ci][:tw, :w],
                                    lhsT=src_sb[:, k, t0:t0 + tw],
                                    rhs=wk_t[:, c0:c0 + w],
                                    start=(k == 0), stop=(k == k_tiles - 1))
                    for (t_i, t0, tw) in grp:
                        evac(t_i, t0, tw, psums[t_i])

            def evac_accum_x(t_i, t0, tw, pss):
                for ci, (c0, w) in enumerate(CH_D):
                    nc.vector.scalar_tensor_tensor(
                        out=x[:tw, t_i, c0:c0 + w],
                        in0=pss[ci][:tw, :w], scalar=1.0,
                        in1=x[:tw, t_i, c0:c0 + w],
                        op0=ALU.mult, op1=ALU.add)

            def evac_v(t_i, t0, tw, pss):
                for ci, (c0, w) in enumerate(CH_D):
                    h0 = c0 // HD
                    nh = w // HD
                    nc.scalar.activation(
                        out=vbuf[:tw, t_i, h0:h0 + nh, 0:HD],
                        in_=pss[ci][:tw, :w].rearrange("p (h d) -> p h d", d=HD),
                        func=AF.Copy)

            def add_bias_to_x(b_dram_l, tiles=None):
                b_bc = lnv.tile([128, D], F32, tag="xb")
                nc.sync.dma_start(out=b_bc[:], in_=_bcast128(b_dram_l))
                for t_i, (t0, tw) in enumerate(TT):
                    if tiles is not None and t_i not in tiles:
                        continue
                    nc.vector.tensor_add(out=x[:tw, t_i, :], in0=x[:tw, t_i, :],
                                         in1=b_bc[:tw])

            def attention(ch, s_tiles=TT):
                """Row-tiled attention over query chunks `ch` (writes OT)."""
                for j in range(ND):
                    ha, hb = 2 * j, 2 * j + 1
                    u_a = upool.tile([128, NT, SPAD], BF16, tag="u", name="ua")
                    u_b = upool.tile([128, NT, SPAD], BF16, tag="u", name="ub")
                    for s_i, (s0, sw) in enumerate(s_tiles):
                        pa = [pp.tile([128, 512], F32, tag="b", name="pa")
                              for _ in ch]
                        pb_ = [pp.tile([128, 512], F32, tag="b", name="pb")
                               for _ in ch]
                        for ci, (c0, w) in enumerate(ch):
                            nc.tensor.matmul(pa[ci][:sw, :w],
                                             lhsT=KT[0:64, j, s0:s0 + sw],
                                             rhs=QT[0:64, j, c0:c0 + w],
                                             start=True, stop=True)
                            nc.tensor.matmul(pb_[ci][:sw, :w],
                                             lhsT=KT[64:128, j, s0:s0 + sw],
                                             rhs=QT[64:128, j, c0:c0 + w],
                                             start=True, stop=True)
                        for ci, (c0, w) in enumerate(ch):
                            nc.scalar.activation(out=u_a[:sw, s_i, c0:c0 + w],
                                                 in_=pa[ci][:sw, :w],
                                                 func=AF.Exp, scale=SCALE)
                            nc.scalar.activation(out=u_b[:sw, s_i, c0:c0 + w],
                                                 in_=pb_[ci][:sw, :w],
                                                 func=AF.Exp, scale=SCALE)

                    # AV + normalization + O^T evac for both heads
                    for h, u_h in ((ha, u_a), (hb, u_b)):
                        r = (h % 2) * 64
                        po = [pp.tile([128, 512], F32, tag="b", name="po")
                              for _ in ch]
                        for s_i, (s0, sw) in enumerate(s_tiles):
                            for ci, (c0, w) in enumerate(ch):
                                nc.tensor.matmul(po[ci][0:HD + 1, :w],
                                                 lhsT=vbuf[:sw, s_i, h, :],
                                                 rhs=u_h[:sw, s_i, c0:c0 + w],
                                                 start=(s_i == 0),
                                                 stop=(s_i == len(s_tiles) - 1))
                        # denominator row -> SBUF (scalar: ACT can read PSUM)
                        drow = rows.tile([128, SPAD], BF16, tag="dr")
                        for ci, (c0, w) in enumerate(ch):
                            nc.scalar.activation(out=drow[0:1, c0:c0 + w],
                                                 in_=po[ci][HD:HD + 1, :w],
                                                 func=AF.Copy)
                        # K=1 bf16 ones-matmul broadcast of the denominator
                        pd = [pp.tile([128, 512], F32, tag="b", name="pd")
                              for _ in ch]
                        for ci, (c0, w) in enumerate(ch):
                            nc.tensor.matmul(pd[ci][0:HD, :w],
                                             lhsT=ones_sb[0:1, 0:HD],
                                             rhs=drow[0:1, c0:c0 + w],
                                             start=True, stop=True)
                        rbc = rows.tile([128, SPAD], F32, tag="rb")
                        for ci, (c0, w) in enumerate(ch):
                            nc.vector.reciprocal(out=rbc[0:HD, c0:c0 + w],
                                                 in_=pd[ci][0:HD, :w])
                        # O^T = po * (1/denom)
                        for ci, (c0, w) in enumerate(ch):
                            nc.vector.tensor_mul(
                                out=OT[r:r + HD, j, c0:c0 + w],
                                in0=po[ci][0:HD, :w],
                                in1=rbc[0:HD, c0:c0 + w])

            # ---------- transformer layers ----------
            for l in range(layers):
                last = cls_last and (l == layers - 1)
                ch_q = CH_CLS if last else CH_T

                layer_norm_into_hT(ln1g[l] if has_gb else None,
                                   ln1b[l] if has_gb else None)

                for n in range(ND):
                    lin_T_n(wk[l], KT, n)
                lin_N(wv[l], hT, ND, evac_v)
                for n in range(ND):
                    lin_T_n(wq[l], QT, n, ch=ch_q)

                attention(ch_q)

                if not last:
                    # proj + residual over all tokens
                    lin_N(pw[l], OT, ND, evac_accum_x)
                    if has_bias:
                        add_bias_to_x(pb[l])
                    layer_norm_into_hT(ln2g[l] if has_gb else None,
                                       ln2b[l] if has_gb else None)
                    if has_bias:
                        fb1_sb = rows.tile([128, NF], F32, tag="fb1")
                        nc.sync.dma_start(out=fb1_sb[:],
                                          in_=fb1[l].rearrange("(t p) -> p t",
                                                               p=128))
                    else:
                        fb1_sb = None
                    for n in range(NF):
                        lin_T_n(fw1[l], h3T, n, relu=True, bias_col=fb1_sb)
                    lin_N(fw2[l], h3T, NF, evac_accum_x)
                    if has_bias:
                        add_bias_to_x(fb2[l])
                else:
                    # cls-only epilogue: proj, LN2, FFN for the cls token.
                    # DVE can't start at partition 76, so bounce the cls row
                    # through partition 0 with SBUF->SBUF DMAs.
                    def evac_cls(t_i, t0, tw, pss):
                        xc = rows.tile([128, D], F32, tag="xc")
                        nc.sync.dma_start(out=xc[0:1, :],
                                          in_=x[CLS_P:CLS_P + 1, CLS_J, :])
                        for ci, (c0, w) in enumerate(CH_D):
                            nc.vector.tensor_add(out=xc[0:1, c0:c0 + w],
                                                 in0=xc[0:1, c0:c0 + w],
                                                 in1=pss[ci][0:1, :w])
                        nc.sync.dma_start(out=x[CLS_P:CLS_P + 1, CLS_J, :],
                                          in_=xc[0:1, :])

                    def lin_cls(w_dram_l, src_sb, k_tiles):
                        """psum[1, 768] = src[:, :, CLS].T @ w ; accum into x."""
                        pss = [pp.tile([128, 512], F32, tag="b", name=f"lc{ci}")
                               for ci in range(len(CH_D))]
                        for k in range(k_tiles):
                            wk_t = rhsk.tile([128, D], BF16, tag="rhsk")
                            nc.gpsimd.dma_start(
                                out=wk_t[:],
                                in_=w_dram_l[k * 128:(k + 1) * 128, :])
                            for ci, (c0, w) in enumerate(CH_D):
                                nc.tensor.matmul(
                                    pss[ci][0:1, :w],
                                    lhsT=src_sb[:, k, CLS:CLS + 1],
                                    rhs=wk_t[:, c0:c0 + w],
                                    start=(k == 0), stop=(k == k_tiles - 1))
                        evac_cls(None, None, None, pss)

                    lin_cls(pw[l], OT, ND)
                    if has_bias:
                        add_bias_to_x(pb[l], tiles=[CLS_J])
                    # LN2 on the cls tile only (row CLS_P of tile CLS_J)
                    layer_norm_into_hT(ln2g[l] if has_gb else None,
                                       ln2b[l] if has_gb else None,
                                       tiles=[CLS_J])
                    if has_bias:
                        fb1_sb = rows.tile([128, NF], F32, tag="fb1")
                        nc.sync.dma_start(out=fb1_sb[:],
                                          in_=fb1[l].rearrange("(t p) -> p t",
                                                               p=128))
                    else:
                        fb1_sb = None
                    for n in range(NF):
                        lin_T_n(fw1[l], h3T, n, relu=True, bias_col=fb1_sb,
                                ch=CH_CLS)
                    lin_cls(fw2[l], h3T, NF)
                    if has_bias:
                        add_bias_to_x(fb2[l], tiles=[CLS_J])

            # ---------- output: cls residual row (row 588 = j4, p76) ----------
            nc.sync.dma_start(out=clsout[:, :], in_=x[CLS_P:CLS_P + 1, CLS_J, :])

    nc.finalize()
    return nc


# ======================= host side =======================

def _sincos_pos(T, d):
    i = np.arange(T, dtype=np.float64)[:, None]
    j = np.arange(d, dtype=np.float64)[None, :]
    je = np.where(j % 2 == 0, j, j - 1)
    ang = i / np.power(10000.0, je / d)
    pe = np.where(j % 2 == 0, np.sin(ang), np.cos(ang))
    return pe.astype(np.float32)


def _patchify_stacked(img):
    b = img.shape[0]
    x = img.reshape(b, IMG // P, P, IMG // P, P, 3, HS)
    x = x.transpose(0, 1, 3, 6, 2, 4, 5)
    return x.reshape(b, NP * HS, P * P * 3)


def _patchify3(img):
    b = img.shape[0]
    x = img.reshape(b, IMG // P, P, IMG // P, P, 3)
    x = x.transpose(0, 1, 3, 2, 4, 5)
    return x.reshape(b, NP, P * P * 3)


def _layernorm_np(v, g, b, eps=1e-5):
    m = v.mean(axis=-1, keepdims=True)
    s = v.var(axis=-1, keepdims=True)
    return (v - m) / np.sqrt(s + eps) * g + b


PERM = np.concatenate([np.arange(2, 394), np.arange(471, 667),
                       np.array([0, 1]), np.arange(394, 471)])


def kernel(**inputs):
    global LAST_EXEC_NS
    f32 = lambda k: np.asarray(inputs[k], dtype=np.float32)
    bf = lambda a: np.ascontiguousarray(np.asarray(a, dtype=np.float32)
                                        .astype(ml_dtypes.bfloat16))

    has_bias = any(np.any(f32(k)) for k in ("proj_b", "ff_b1", "ff_b2"))
    has_gb = (np.any(f32("ln1_g") != 1.0) or np.any(f32("ln1_b")) or
              np.any(f32("ln2_g") != 1.0) or np.any(f32("ln2_b")))

    key = (has_gb, has_bias, CLS_LAST)
    if key not in _CACHE:
        _CACHE[key] = build_nc(has_gb=has_gb, has_bias=has_bias,
                               cls_last=CLS_LAST)
    nc = _CACHE[key]

    images = f32("images")
    goal_imgs = f32("goal_imgs")
    pose = f32("pose")
    txt = np.asarray(inputs["goals_txt"]).astype(np.int64)
    tok_emb = f32("tok_emb")

    # pose MLP (host, exact fp32 - 4.7 MFLOP)
    pose_tok = np.maximum(pose @ f32("pose_w1") + f32("pose_b1"), 0.0) \
        @ f32("pose_w2") + f32("pose_b2")                       # [B, D]

    pos = _sincos_pos(SEQ, D)                                    # [667, D]
    content = np.zeros((B, SEQ, D), np.float32)
    content[:, 0, :] = f32("cls_tok")[0, 0]
    content[:, 1, :] = pose_tok
    content[:, 2:394, :] = f32("obs_b")
    content[:, 394:471, :] = tok_emb[txt]
    content[:, 471:667, :] = f32("goal_b")
    base = (content + pos[None])[:, PERM, :]                     # permuted
    base_pad = np.zeros((B, TPAD, D), np.float32)
    base_pad[:, :SEQ, :] = base

    p_obs = _patchify_stacked(images)                            # [B, 392, 768]
    p_goal = _patchify3(goal_imgs)                               # [B, 196, 768]
    pobsT = bf(p_obs.transpose(0, 2, 1))                         # [B, 768, 392]
    pgoalT_np = np.zeros((B, D, 204), np.float32)
    pgoalT_np[:, :, 8:] = p_goal.transpose(0, 2, 1)
    pgoalT = bf(pgoalT_np)

    shared = {
        "obs_w": bf(f32("obs_w")), "goal_w": bf(f32("goal_w")),
        "wq": bf(f32("wq")), "wk": bf(f32("wk")), "wv": bf(f32("wv")),
        "pw": bf(f32("proj_w")), "fw1": bf(f32("ff_w1")), "fw2": bf(f32("ff_w2")),
    }
    if has_bias:
        shared.update({"pb": f32("proj_b"), "fb1": f32("ff_b1"),
                       "fb2": f32("ff_b2")})
    if has_gb:
        shared.update({"ln1g": f32("ln1_g"), "ln1b": f32("ln1_b"),
                       "ln2g": f32("ln2_g"), "ln2b": f32("ln2_b")})
    in_maps = []
    for b in range(B):
        m = dict(shared)
        m["base"] = np.ascontiguousarray(base_pad[b])
        m["pobsT"] = np.ascontiguousarray(pobsT[b])
        m["pgoalT"] = np.ascontiguousarray(pgoalT[b])
        in_maps.append(m)

    res = run_bass_kernel_spmd(nc, in_maps, list(range(B)), trace=TRACE,
                               trace_cores=TRACE_CORES if TRACE else None)
    LAST_EXEC_NS = res.exec_time_ns

    cls = np.stack([np.asarray(res.results[b]["clsout"][0], np.float32)
                    for b in range(B)])                          # [B, D]
    h = _layernorm_np(cls, f32("lnf_g"), f32("lnf_b"))
    h = _layernorm_np(h, f32("hln_g"), f32("hln_b"))
    out = h @ f32("head_w") + f32("head_b")
    return out.astype(np.float32)


# revision 18
# speedup vs baseline: 1.4741x; 1.1339x over previous
"""Trainium2 Bass kernel: ViT-style multimodal transformer (12L, D=768, H=12).

Strategy: pure data parallel - 8 batch elements, one per NeuronCore.
Each core runs the full transformer on its [667, 768] token sequence.

Optimizations over the v0 baseline (all bf16 - fp8 fails the 2e-2 gate):
  - Attention S^T computed as row-tiled head PAIRS: even head uses PE rows
    0-63, odd head rows 64-127 (tile_position auto-derived from
    base_partition), so the two K=64 matmuls run concurrently -> S^T cost
    halves.
  - Softmax denominator (ones-column of V) broadcast with a K=1 BF16
    ones-matmul of the raw denominator row (the baseline broadcast fp32
    reciprocals), then a 64-lane reciprocal; normalization fused into the
    O^T evacuation (one scalar_tensor_tensor op). Kills the 2.2us
    single-partition reciprocals of the baseline.
  - Residual adds fused: x = psum + x in one DVE op (no separate copies,
    no K=1 fp32 bias matmuls; biases/LN gains only emitted when nonzero).
  - Last layer computes attention/proj/FFN for the cls token only.

Token order is permuted (attention is permutation-equivariant; positional
embeddings are baked into the additive base): [obs(392) | goal(196) | cls |
pose | text(77)], so patch embeddings land partition-aligned. cls lives at
row 588 = (j=4, p=76).
"""

import numpy as np
import ml_dtypes

import concourse.bass as bass
import concourse.bacc as bacc_mod
import concourse.mybir as mybir
import concourse.tile as tile
from concourse.bass_utils import run_bass_kernel_spmd
from concourse.masks import make_identity

BF16 = mybir.dt.bfloat16
F32 = mybir.dt.float32
AF = mybir.ActivationFunctionType
ALU = mybir.AluOpType

L, H, D, HD = 12, 12, 768, 64
P, IMG, NP, HS = 16, 224, 196, 2
TBLK, VOCAB, POSE_DIM, OUT = 77, 96, 7, 7
B = 8
SEQ = 667          # 1 cls + 1 pose + 392 obs + 77 text + 196 goal
TPAD = 768         # padded token slots (6 partition tiles)
SPAD = 672         # padded free-dim length of transposed activations
NT = 6             # token partition tiles
ND = 6             # feature partition tiles (768/128)
NF = 24            # ffn feature tiles (3072/128)
SCALE = float(D) ** -0.5
EPS = 1e-5
CLS = 588          # permuted cls position = (tile 4, row 76)
CLS_J, CLS_P = 4, 76

# token tiles (start, width)
TT = [(0, 128), (128, 128), (256, 128), (384, 128), (512, 128), (640, 27)]
CH_T = [(0, 512), (512, 155)]   # SEQ chunks (psum bank = 512 fp32)
CH_D = [(0, 512), (512, 256)]   # D chunks
CH_CLS = [(CLS, 1)]             # cls-only chunk (last layer)

# Runtime knobs (test.py may flip these)
TRACE = False
TRACE_CORES = [0]
CLS_LAST = True
LAST_EXEC_NS = None
_CACHE = {}


def _bcast128(ap1d):
    """DMA access pattern broadcasting a 1-D DRAM row across 128 partitions."""
    return bass.AP(tensor=ap1d.tensor, offset=ap1d.offset,
                   ap=[[0, 128]] + list(ap1d.ap))


def build_nc(has_gb=False, has_bias=False, layers=L, cls_last=True):
    nc = bacc_mod.Bacc()

    # ---- per-core data inputs ----
    base = nc.declare_dram_parameter("base", [TPAD, D], F32, isOutput=False)
    pobsT = nc.declare_dram_parameter("pobsT", [D, 392], BF16, isOutput=False)
    pgoalT = nc.declare_dram_parameter("pgoalT", [D, 204], BF16, isOutput=False)
    # ---- shared weights ----
    obs_w = nc.declare_dram_parameter("obs_w", [D, D], BF16, isOutput=False)
    goal_w = nc.declare_dram_parameter("goal_w", [D, D], BF16, isOutput=False)
    wq = nc.declare_dram_parameter("wq", [L, D, D], BF16, isOutput=False)
    wk = nc.declare_dram_parameter("wk", [L, D, D], BF16, isOutput=False)
    wv = nc.declare_dram_parameter("wv", [L, D, D], BF16, isOutput=False)
    pw = nc.declare_dram_parameter("pw", [L, D, D], BF16, isOutput=False)
    fw1 = nc.declare_dram_parameter("fw1", [L, D, 4 * D], BF16, isOutput=False)
    fw2 = nc.declare_dram_parameter("fw2", [L, 4 * D, D], BF16, isOutput=False)
    if has_bias:
        pb = nc.declare_dram_parameter("pb", [L, D], F32, isOutput=False)
        fb1 = nc.declare_dram_parameter("fb1", [L, 4 * D], F32, isOutput=False)
        fb2 = nc.declare_dram_parameter("fb2", [L, D], F32, isOutput=False)
    if has_gb:
        ln1g = nc.declare_dram_parameter("ln1g", [L, D], F32, isOutput=False)
        ln1b = nc.declare_dram_parameter("ln1b", [L, D], F32, isOutput=False)
        ln2g = nc.declare_dram_parameter("ln2g", [L, D], F32, isOutput=False)
        ln2b = nc.declare_dram_parameter("ln2b", [L, D], F32, isOutput=False)
    clsout = nc.declare_dram_parameter("clsout", [1, D], F32, isOutput=True)

    with tile.TileContext(nc) as tc:
        with (
            tc.tile_pool(name="singles", bufs=1) as singles,
            tc.tile_pool(name="wblk", bufs=4) as wblk,    # lin_T weight tiles
            tc.tile_pool(name="rhsk", bufs=6) as rhsk,    # lin_N weight k-tiles
            tc.tile_pool(name="upool", bufs=4) as upool,  # exp(S^T) per head
            tc.tile_pool(name="hn", bufs=2) as hn,
            tc.tile_pool(name="rows", bufs=2) as rows,
            tc.tile_pool(name="stats", bufs=6) as stats,
            tc.tile_pool(name="lnv", bufs=4) as lnv,
            tc.tile_pool(name="pp", bufs=8, space="PSUM") as pp,
        ):
            # ---------- persistent SBUF ----------
            ident = singles.tile([128, 128], BF16)
            make_identity(nc, ident)
            eps_sb = singles.tile([128, 1], F32)
            nc.vector.memset(eps_sb, EPS)
            ones_sb = singles.tile([1, 128], BF16)
            nc.vector.memset(ones_sb, 1.0)

            x = singles.tile([128, NT, D], F32)            # residual stream
            hT = singles.tile([128, ND, SPAD], BF16)       # LN output, transposed
            QT = singles.tile([128, ND, SPAD], BF16)
            KT = singles.tile([128, ND, SPAD], BF16)
            vbuf = singles.tile([128, NT, H, HD + 1], BF16)  # V natural + ones col
            OT = singles.tile([128, ND, SPAD], BF16)       # attn out, transposed
            h3T = singles.tile([128, NF, SPAD], BF16)      # relu ffn hidden, transposed

            nc.vector.memset(vbuf[:, :, :, HD:HD + 1], 1.0)

            # ---------- load residual base ----------
            nc.sync.dma_start(out=x[:], in_=base.rearrange("(j p) d -> p j d", p=128))

            # ---------- patch embeddings ----------
            pobs_sb = singles.tile([128, ND, 392], BF16)
            nc.sync.dma_start(out=pobs_sb[:],
                              in_=pobsT.rearrange("(kt kp) t -> kp kt t", kp=128))
            pgoal_sb = singles.tile([128, ND, 204], BF16)
            nc.sync.dma_start(out=pgoal_sb[:],
                              in_=pgoalT.rearrange("(kt kp) t -> kp kt t", kp=128))

            def embed_add(psrc_sb, w_dram, ptiles, dests):
                # ptiles: list of (col0, width); dests: list of (xrow0, xj)
                for gi in range(0, len(ptiles), 2):
                    grp = list(range(gi, min(gi + 2, len(ptiles))))
                    psums = {}
                    for t_i in grp:
                        psums[t_i] = [pp.tile([128, 512], F32, tag="b",
                                              name=f"pe{t_i}{ci}")
                                      for ci in range(len(CH_D))]
                    for k in range(ND):
                        wk_t = rhsk.tile([128, D], BF16, tag="rhsk")
                        nc.gpsimd.dma_start(out=wk_t[:],
                                            in_=w_dram[k * 128:(k + 1) * 128, :])
                        for t_i in grp:
                            c0, cw = ptiles[t_i]
                            for ci, (s, w) in enumerate(CH_D):
                                nc.tensor.matmul(
                                    psums[t_i][ci][:cw, :w],
                                    lhsT=psrc_sb[:, k, c0:c0 + cw],
                                    rhs=wk_t[:, s:s + w],
                                    start=(k == 0), stop=(k == ND - 1))
                    for t_i in grp:
                        c0, cw = ptiles[t_i]
                        r0, xj = dests[t_i]
                        for ci, (s, w) in enumerate(CH_D):
                            nc.vector.tensor_add(
                                out=x[r0:r0 + cw, xj, s:s + w],
                                in0=x[r0:r0 + cw, xj, s:s + w],
                                in1=psums[t_i][ci][:cw, :w])

            embed_add(pobs_sb, obs_w,
                      [(0, 128), (128, 128), (256, 128), (384, 8)],
                      [(0, 0), (0, 1), (0, 2), (0, 3)])
            embed_add(pgoal_sb, goal_w,
                      [(0, 128), (128, 76)],
                      [(0, 3), (0, 4)])

            # ---------- helpers ----------
            def layer_norm_tile(ti, t0, tw, g_bc, b_bc):
                st = stats.tile([128, 3, 6], F32, tag="bnst")
                mv = stats.tile([128, 2], F32, tag="bnmv")
                rstd = stats.tile([128, 1], F32, tag="rstd")
                xi = x[:tw, ti, :].rearrange("p (s c) -> p s c", s=3)
                for s in range(3):
                    nc.vector.bn_stats(out=st[:tw, s, :], in_=xi[:, s, :])
                nc.vector.bn_aggr(out=mv[:tw], in_=st[:tw])
                nc.scalar.activation(out=rstd[:tw], in_=mv[:tw, 1:2],
                                     func=AF.Sqrt, bias=eps_sb[:tw], scale=1.0)
                nc.vector.reciprocal(out=rstd[:tw], in_=rstd[:tw])
                hnat = hn.tile([128, D], BF16, tag="hnat")
                nc.vector.tensor_scalar(out=hnat[:tw], in0=x[:tw, ti, :],
                                        scalar1=mv[:tw, 0:1], scalar2=rstd[:tw],
                                        op0=ALU.subtract, op1=ALU.mult)
                if has_gb:
                    nc.vector.tensor_mul(out=hnat[:tw], in0=hnat[:tw], in1=g_bc[:tw])
                    nc.vector.tensor_add(out=hnat[:tw], in0=hnat[:tw], in1=b_bc[:tw])
                # transpose into hT
                for dj in range(ND):
                    pt = pp.tile([128, 128], BF16, tag="b", name="pt")
                    nc.tensor.transpose(pt[:, :tw],
                                        hnat[:tw, dj * 128:(dj + 1) * 128],
                                        ident[:tw, :tw])
                    nc.scalar.activation(out=hT[:, dj, t0:t0 + tw],
                                         in_=pt[:, :tw], func=AF.Copy)

            def layer_norm_into_hT(g_dram=None, b_dram=None, tiles=None):
                g_bc = b_bc = None
                if has_gb:
                    g_bc = lnv.tile([128, D], F32, tag="g")
                    b_bc = lnv.tile([128, D], F32, tag="bb")
                    nc.sync.dma_start(out=g_bc[:], in_=_bcast128(g_dram))
                    nc.sync.dma_start(out=b_bc[:], in_=_bcast128(b_dram))
                for ti, (t0, tw) in enumerate(TT):
                    if tiles is not None and ti not in tiles:
                        continue
                    layer_norm_tile(ti, t0, tw, g_bc, b_bc)

            def lin_T_n(w_dram_l, out_sb, n, src=None, relu=False, bias_col=None,
                        ch=CH_T):
                """One n-tile of a transposed-output linear: out[:, n, t]."""
                if src is None:
                    src = hT
                k_tiles = src.shape[1]
                wb = wblk.tile([128, k_tiles, 128], BF16, tag="wblk")
                nc.gpsimd.dma_start(
                    out=wb[:],
                    in_=w_dram_l.rearrange("(kt kp) n -> kp kt n", kp=128)
                    [:, :, n * 128:(n + 1) * 128])
                pss = [pp.tile([128, 512], F32, tag="b", name=f"lt{ci}")
                       for ci in range(len(ch))]
                for k in range(k_tiles):
                    for ci, (c0, w) in enumerate(ch):
                        nc.tensor.matmul(
                            pss[ci][:, :w],
                            lhsT=wb[:, k, :],
                            rhs=src[:, k, c0:c0 + w],
                            start=(k == 0), stop=(k == k_tiles - 1))
                for ci, (c0, w) in enumerate(ch):
                    if relu:
                        bias = bias_col[:, n:n + 1] if bias_col is not None else 0.0
                        nc.scalar.activation(out=out_sb[:, n, c0:c0 + w],
                                             in_=pss[ci][:, :w], func=AF.Relu,
                                             bias=bias, scale=1.0)
                    else:
                        nc.scalar.activation(out=out_sb[:, n, c0:c0 + w],
                                             in_=pss[ci][:, :w], func=AF.Copy)

            def lin_N(w_dram_l, src_sb, k_tiles, evac, tiles=None):
                """Natural-layout output: psum[t, 0:768] = src.T @ w per token tile."""
                tlist = [(t_i, t0, tw) for t_i, (t0, tw) in enumerate(TT)
                         if tiles is None or t_i in tiles]
                for gi in range(0, len(tlist), 2):
                    grp = tlist[gi:gi + 2]
                    psums = {}
                    for (t_i, t0, tw) in grp:
                        psums[t_i] = [pp.tile([128, 512], F32, tag="b",
                                              name=f"ln{t_i}{ci}")
                                      for ci in range(len(CH_D))]
                    for k in range(k_tiles):
                        wk_t = rhsk.tile([128, D], BF16, tag="rhsk")
                        nc.gpsimd.dma_start(out=wk_t[:],
                                            in_=w_dram_l[k * 128:(k + 1) * 128, :])
                        for (t_i, t0, tw) in grp:
                            for ci, (c0, w) in enumerate(CH_D):
                                nc.tensor.matmul(
                                    psums[t_i][ci][:tw, :w],
                                    lhsT=src_sb[:, k, t0:t0 + tw],
                                    rhs=wk_t[:, c0:c0 + w],
                                    start=(k == 0), stop=(k == k_tiles - 1))
                    for (t_i, t0, tw) in grp:
                        evac(t_i, t0, tw, psums[t_i])

            def evac_accum_x(t_i, t0, tw, pss):
                for ci, (c0, w) in enumerate(CH_D):
                    nc.vector.scalar_tensor_tensor(
                        out=x[:tw, t_i, c0:c0 + w],
                        in0=pss[ci][:tw, :w], scalar=1.0,
                        in1=x[:tw, t_i, c0:c0 + w],
                        op0=ALU.mult, op1=ALU.add)

            def evac_v(t_i, t0, tw, pss):
                for ci, (c0, w) in enumerate(CH_D):
                    h0 = c0 // HD
                    nh = w // HD
                    nc.vector.tensor_copy(
                        out=vbuf[:tw, t_i, h0:h0 + nh, 0:HD],
                        in_=pss[ci][:tw, :w].rearrange("p (h d) -> p h d", d=HD))

            def add_bias_to_x(b_dram_l, tiles=None):
                b_bc = lnv.tile([128, D], F32, tag="xb")
                nc.sync.dma_start(out=b_bc[:], in_=_bcast128(b_dram_l))
                for t_i, (t0, tw) in enumerate(TT):
                    if tiles is not None and t_i not in tiles:
                        continue
                    nc.vector.tensor_add(out=x[:tw, t_i, :], in0=x[:tw, t_i, :],
                                         in1=b_bc[:tw])

            def attention(ch, s_tiles=TT):
                """Row-tiled attention over query chunks `ch` (writes OT).

                Software-pipelined by one head pair: the S^T matmuls of pair
                j+1 are interleaved with AV/normalization of pair j, so the
                tensor engine computes S^T of the next pair while the scalar
                engine runs exp of the current one. The interleave order also
                keeps the 8-slot PSUM FIFO rotation deadlock-free: every
                slot's consumer is emitted within 8 allocations.
                """
                def emit_st_si(j, s_i, u_a, u_b):
                    s0, sw = s_tiles[s_i]
                    pa = [pp.tile([128, 512], F32, tag="b", name="pa")
                          for _ in ch]
                    pb_ = [pp.tile([128, 512], F32, tag="b", name="pb")
                           for _ in ch]
                    for ci, (c0, w) in enumerate(ch):
                        nc.tensor.matmul(pa[ci][:sw, :w],
                                         lhsT=KT[0:64, j, s0:s0 + sw],
                                         rhs=QT[0:64, j, c0:c0 + w],
                                         start=True, stop=True)
                        nc.tensor.matmul(pb_[ci][:sw, :w],
                                         lhsT=KT[64:128, j, s0:s0 + sw],
                                         rhs=QT[64:128, j, c0:c0 + w],
                                         start=True, stop=True)
                    for ci, (c0, w) in enumerate(ch):
                        nc.scalar.activation(out=u_a[:sw, s_i, c0:c0 + w],
                                             in_=pa[ci][:sw, :w],
                                             func=AF.Exp, scale=SCALE)
                        nc.scalar.activation(out=u_b[:sw, s_i, c0:c0 + w],
                                             in_=pb_[ci][:sw, :w],
                                             func=AF.Exp, scale=SCALE)

                def emit_avmm(j, h, u_h):
                    """AV matmuls for one head + denominator row copy."""
                    po = [pp.tile([128, 512], F32, tag="b", name="po")
                          for _ in ch]
                    for s_i, (s0, sw) in enumerate(s_tiles):
                        for ci, (c0, w) in enumerate(ch):
                            nc.tensor.matmul(po[ci][0:HD + 1, :w],
                                             lhsT=vbuf[:sw, s_i, h, :],
                                             rhs=u_h[:sw, s_i, c0:c0 + w],
                                             start=(s_i == 0),
                                             stop=(s_i == len(s_tiles) - 1))
                    drow = rows.tile([128, SPAD], BF16, tag="dr")
                    for ci, (c0, w) in enumerate(ch):
                        nc.scalar.activation(out=drow[0:1, c0:c0 + w],
                                             in_=po[ci][HD:HD + 1, :w],
                                             func=AF.Copy)
                    return po, drow

                def emit_norm(j, h, po, drow):
                    """Broadcast 1/denominator and evacuate O^T for one head."""
                    r = (h % 2) * 64
                    pd = [pp.tile([128, 512], F32, tag="b", name="pd")
                          for _ in ch]
                    for ci, (c0, w) in enumerate(ch):
                        nc.tensor.matmul(pd[ci][0:HD, :w],
                                         lhsT=ones_sb[0:1, 0:HD],
                                         rhs=drow[0:1, c0:c0 + w],
                                         start=True, stop=True)
                    rbc = rows.tile([128, SPAD], F32, tag="rb")
                    for ci, (c0, w) in enumerate(ch):
                        nc.vector.reciprocal_approx_fast(
                            out=rbc[0:HD, c0:c0 + w], in_=pd[ci][0:HD, :w])
                    for ci, (c0, w) in enumerate(ch):
                        nc.vector.tensor_mul(
                            out=OT[r:r + HD, j, c0:c0 + w],
                            in0=po[ci][0:HD, :w],
                            in1=rbc[0:HD, c0:c0 + w])

                def alloc_u():
                    u_a = upool.tile([128, NT, SPAD], BF16, tag="u", name="ua")
                    u_b = upool.tile([128, NT, SPAD], BF16, tag="u", name="ub")
                    return u_a, u_b

                ns = len(s_tiles)
                u_prev = alloc_u()
                for s_i in range(ns):
                    emit_st_si(0, s_i, *u_prev)
                for j in range(ND):
                    nxt = j + 1 < ND
                    if nxt:
                        u_cur = alloc_u()
                        emit_st_si(j + 1, 0, *u_cur)
                        emit_st_si(j + 1, 1, *u_cur)
                    po_a, dr_a = emit_avmm(j, 2 * j, u_prev[0])
                    if nxt:
                        emit_st_si(j + 1, 2, *u_cur)
                    emit_norm(j, 2 * j, po_a, dr_a)
                    if nxt:
                        emit_st_si(j + 1, 3, *u_cur)
                    po_b, dr_b = emit_avmm(j, 2 * j + 1, u_prev[1])
                    if nxt:
                        emit_st_si(j + 1, 4, *u_cur)
                    emit_norm(j, 2 * j + 1, po_b, dr_b)
                    if nxt:
                        emit_st_si(j + 1, ns - 1, *u_cur)
                        u_prev = u_cur

            # ---------- transformer layers ----------
            for l in range(layers):
                last = cls_last and (l == layers - 1)
                ch_q = CH_CLS if last else CH_T

                layer_norm_into_hT(ln1g[l] if has_gb else None,
                                   ln1b[l] if has_gb else None)

                for n in range(ND):
                    lin_T_n(wk[l], KT, n)
                lin_N(wv[l], hT, ND, evac_v)
                for n in range(ND):
                    lin_T_n(wq[l], QT, n, ch=ch_q)

                attention(ch_q)

                if not last:
                    # proj + residual over all tokens
                    lin_N(pw[l], OT, ND, evac_accum_x)
                    if has_bias:
                        add_bias_to_x(pb[l])
                    layer_norm_into_hT(ln2g[l] if has_gb else None,
                                       ln2b[l] if has_gb else None)
                    if has_bias:
                        fb1_sb = rows.tile([128, NF], F32, tag="fb1")
                        nc.sync.dma_start(out=fb1_sb[:],
                                          in_=fb1[l].rearrange("(t p) -> p t",
                                                               p=128))
                    else:
                        fb1_sb = None
                    for n in range(NF):
                        lin_T_n(fw1[l], h3T, n, relu=True, bias_col=fb1_sb)
                    lin_N(fw2[l], h3T, NF, evac_accum_x)
                    if has_bias:
                        add_bias_to_x(fb2[l])
                else:
                    # cls-only epilogue: proj, LN2, FFN for the cls token.
                    # DVE can't start at partition 76, so bounce the cls row
                    # through partition 0 with SBUF->SBUF DMAs.
                    def evac_cls(t_i, t0, tw, pss):
                        xc = rows.tile([128, D], F32, tag="xc")
                        nc.sync.dma_start(out=xc[0:1, :],
                                          in_=x[CLS_P:CLS_P + 1, CLS_J, :])
                        for ci, (c0, w) in enumerate(CH_D):
                            nc.vector.tensor_add(out=xc[0:1, c0:c0 + w],
                                                 in0=xc[0:1, c0:c0 + w],
                                                 in1=pss[ci][0:1, :w])
                        nc.sync.dma_start(out=x[CLS_P:CLS_P + 1, CLS_J, :],
                                          in_=xc[0:1, :])

                    def lin_cls(w_dram_l, src_sb, k_tiles):
                        """psum[1, 768] = src[:, :, CLS].T @ w ; accum into x."""
                        pss = [pp.tile([128, 512], F32, tag="b", name=f"lc{ci}")
                               for ci in range(len(CH_D))]
                        for k in range(k_tiles):
                            wk_t = rhsk.tile([128, D], BF16, tag="rhsk")
                            nc.gpsimd.dma_start(
                                out=wk_t[:],
                                in_=w_dram_l[k * 128:(k + 1) * 128, :])
                            for ci, (c0, w) in enumerate(CH_D):
                                nc.tensor.matmul(
                                    pss[ci][0:1, :w],
                                    lhsT=src_sb[:, k, CLS:CLS + 1],
                                    rhs=wk_t[:, c0:c0 + w],
                                    start=(k == 0), stop=(k == k_tiles - 1))
                        evac_cls(None, None, None, pss)

                    lin_cls(pw[l], OT, ND)
                    if has_bias:
                        add_bias_to_x(pb[l], tiles=[CLS_J])
                    # LN2 on the cls tile only (row CLS_P of tile CLS_J)
                    layer_norm_into_hT(ln2g[l] if has_gb else None,
                                       ln2b[l] if has_gb else None,
                                       tiles=[CLS_J])
                    if has_bias:
                        fb1_sb = rows.tile([128, NF], F32, tag="fb1")
                        nc.sync.dma_start(out=fb1_sb[:],
                                          in_=fb1[l].rearrange("(t p) -> p t",
                                                               p=128))
                    else:
                        fb1_sb = None
                    for n in range(NF):
                        lin_T_n(fw1[l], h3T, n, relu=True, bias_col=fb1_sb,
                                ch=CH_CLS)
                    lin_cls(fw2[l], h3T, NF)
                    if has_bias:
                        add_bias_to_x(fb2[l], tiles=[CLS_J])

            # ---------- output: cls residual row (row 588 = j4, p76) ----------
            nc.sync.dma_start(out=clsout[:, :], in_=x[CLS_P:CLS_P + 1, CLS_J, :])

    nc.finalize()
    return nc


# ======================= host side =======================

def _sincos_pos(T, d):
    i = np.arange(T, dtype=np.float64)[:, None]
    j = np.arange(d, dtype=np.float64)[None, :]
    je = np.where(j % 2 == 0, j, j - 1)
    ang = i / np.power(10000.0, je / d)
    pe = np.where(j % 2 == 0, np.sin(ang), np.cos(ang))
    return pe.astype(np.float32)


def _patchify_stacked(img):
    b = img.shape[0]
    x = img.reshape(b, IMG // P, P, IMG // P, P, 3, HS)
    x = x.transpose(0, 1, 3, 6, 2, 4, 5)
    return x.reshape(b, NP * HS, P * P * 3)


def _patchify3(img):
    b = img.shape[0]
    x = img.reshape(b, IMG // P, P, IMG // P, P, 3)
    x = x.transpose(0, 1, 3, 2, 4, 5)
    return x.reshape(b, NP, P * P * 3)


def _layernorm_np(v, g, b, eps=1e-5):
    m = v.mean(axis=-1, keepdims=True)
    s = v.var(axis=-1, keepdims=True)
    return (v - m) / np.sqrt(s + eps) * g + b


PERM = np.concatenate([np.arange(2, 394), np.arange(471, 667),
                       np.array([0, 1]), np.arange(394, 471)])


def kernel(**inputs):
    global LAST_EXEC_NS
    f32 = lambda k: np.asarray(inputs[k], dtype=np.float32)
    bf = lambda a: np.ascontiguousarray(np.asarray(a, dtype=np.float32)
                                        .astype(ml_dtypes.bfloat16))

    has_bias = any(np.any(f32(k)) for k in ("proj_b", "ff_b1", "ff_b2"))
    has_gb = (np.any(f32("ln1_g") != 1.0) or np.any(f32("ln1_b")) or
              np.any(f32("ln2_g") != 1.0) or np.any(f32("ln2_b")))

    key = (has_gb, has_bias, CLS_LAST)
    if key not in _CACHE:
        _CACHE[key] = build_nc(has_gb=has_gb, has_bias=has_bias,
                               cls_last=CLS_LAST)
    nc = _CACHE[key]

    images = f32("images")
    goal_imgs = f32("goal_imgs")
    pose = f32("pose")
    txt = np.asarray(inputs["goals_txt"]).astype(np.int64)
    tok_emb = f32("tok_emb")

    # pose MLP (host, exact fp32 - 4.7 MFLOP)
    pose_tok = np.maximum(pose @ f32("pose_w1") + f32("pose_b1"), 0.0) \
        @ f32("pose_w2") + f32("pose_b2")                       # [B, D]

    pos = _sincos_pos(SEQ, D)                                    # [667, D]
    content = np.zeros((B, SEQ, D), np.float32)
    content[:, 0, :] = f32("cls_tok")[0, 0]
    content[:, 1, :] = pose_tok
    content[:, 2:394, :] = f32("obs_b")
    content[:, 394:471, :] = tok_emb[txt]
    content[:, 471:667, :] = f32("goal_b")
    base = (content + pos[None])[:, PERM, :]                     # permuted
    base_pad = np.zeros((B, TPAD, D), np.float32)
    base_pad[:, :SEQ, :] = base

    p_obs = _patchify_stacked(images)                            # [B, 392, 768]
    p_goal = _patchify3(goal_imgs)                               # [B, 196, 768]
    pobsT = bf(p_obs.transpose(0, 2, 1))                         # [B, 768, 392]
    pgoalT_np = np.zeros((B, D, 204), np.float32)
    pgoalT_np[:, :, 8:] = p_goal.transpose(0, 2, 1)
    pgoalT = bf(pgoalT_np)

    shared = {
        "obs_w": bf(f32("obs_w")), "goal_w": bf(f32("goal_w")),
        "wq": bf(f32("wq")), "wk": bf(f32("wk")), "wv": bf(f32("wv")),
        "pw": bf(f32("proj_w")), "fw1": bf(f32("ff_w1")), "fw2": bf(f32("ff_w2")),
    }
    if has_bias:
        shared.update({"pb": f32("proj_b"), "fb1": f32("ff_b1"),
                       "fb2": f32("ff_b2")})
    if has_gb:
        shared.update({"ln1g": f32("ln1_g"), "ln1b": f32("ln1_b"),
                       "ln2g": f32("ln2_g"), "ln2b": f32("ln2_b")})
    in_maps = []
    for b in range(B):
        m = dict(shared)
        m["base"] = np.ascontiguousarray(base_pad[b])
        m["pobsT"] = np.ascontiguousarray(pobsT[b])
        m["pgoalT"] = np.ascontiguousarray(pgoalT[b])
        in_maps.append(m)

    res = run_bass_kernel_spmd(nc, in_maps, list(range(B)), trace=TRACE,
                               trace_cores=TRACE_CORES if TRACE else None)
    LAST_EXEC_NS = res.exec_time_ns

    cls = np.stack([np.asarray(res.results[b]["clsout"][0], np.float32)
                    for b in range(B)])                          # [B, D]
    h = _layernorm_np(cls, f32("lnf_g"), f32("lnf_b"))
    h = _layernorm_np(h, f32("hln_g"), f32("hln_b"))
    out = h @ f32("head_w") + f32("head_b")
    return out.astype(np.float32)
